# revision 33
# baseline (speedup 1.0000x reference)
# DiabaticReadout forward on Trainium2 (Bass/Tile), 8-core data-parallel.
#
# Per sample i: H = [[d0, lam], [lam, d1]] (2x2 symmetric).  Eigenvalues in
# closed form:
#   h = 0.5*(d0+d1);  r = sqrt(0.25*(d0-d1)^2 + lam^2);  e0, e1 = h -/+ r
# (ascending, matches eigh).
#
# Purely elementwise and HBM-bound, so the whole game is bytes: the harness
# gate is rel-err < 2e-2 against a ~7 output scale, while fp16 rounding of
# the streams costs ~1e-3 worst case.  Stream everything as fp16: 6 B/sample
# in + 4 B/sample out = 12.5 MB per core instead of 25 MB, a 2x cut in HBM
# traffic against the ~358 GB/s per-core HBM limit (~35 us floor).
#
# Layout: the host packs the three inputs tile-interleaved into ONE tensor
# ([d0-block | d1-block | lam-block] per [128, F] tile) and both outputs
# come back in one ([e0-block | e1-block]).  One dma_start per tile per
# direction with 12KB/8KB per-partition lines keeps the SDMA engines in
# their high-efficiency regime (separate fp16 tensors gave 4KB lines and
# 5 issues/tile) and the host does pure reshuffling.  d0,d1 are pre-scaled
# by 0.5 during the fp16 cast (a free quantization scale) so no on-device
# halving op is needed.
#
# Engine budget per [128, 2048] tile (~7.3 us of DMA, the pacer; ops run
# ~15% over their cost-model time when the SDMA engines are at full rate):
#   DVE    dif=d0h-d1h, h=d0h+d1h, e0=h-r, e1=h+r  (4 TT @ 2x fp16 ~5.4us)
#   ACT    d2=Square(dif), r=Sqrt(s_psum)          (2 passes ~4.5us)
#   GPSIMD l2=lam*lam (TT ~4us) + the output-store issue (SWDGE ring)
#   PE     s = I.d2 + I.l2 accumulated into PSUM f32 (8 id-matmuls ~4us);
#          the identity weights ship as a tiny extra input
#   Sync   the input-load issue (SP HWDGE ring)
# scalar_tensor_tensor is avoided (only a 1x DVE uop; plain tensor_tensor
# runs 2x on fp16) and both ACT functions live in the single
# sqrt_and_others table so there is exactly one ACT_TABLE_LOAD.
#
# The Tile scheduler keeps per-engine program order, so emitting a tile's
# whole chain at once would make the in-order ACT/DVE streams block
# mid-chain on the PE adder / GPSIMD's lam^2 every tile.  Each Python
# iteration instead emits a 3-stage software pipeline
#   A(i): load, dif/h, d2, l2    B(i-1): PE matmul-add    C(i-2): sqrt,
#   e0/e1, store
# so every op's inputs finished a full tile-period earlier and no engine
# ever blocks inside its stream.

import numpy as np

import concourse.bacc as bacc_mod
import concourse.tile as tile
from concourse import bacc, mybir
from concourse.bass_utils import run_bass_kernel_spmd

import contextlib


@contextlib.contextmanager
def _pin_act_table(keep="sqrt_and_others"):
    """Square and Sqrt both live in the `sqrt_and_others` set, but the
    table-load pass greedily picks the first set containing each function,
    which alternates tables per tile (~2.5us/tile of ACT_TABLE_LOAD
    thrash).  Present every other set as empty during compile so the pass
    pins everything to one table; indices stay aligned with act_info.json."""
    orig = bacc_mod.get_activation_tables

    def patched(arch):
        t = orig(arch)
        assert keep in t, sorted(t)
        return {name: (funcs if name == keep else set()) for name, funcs in t.items()}

    bacc_mod.get_activation_tables = patched
    try:
        yield
    finally:
        bacc_mod.get_activation_tables = orig

N_CORES = 8
P = 128  # SBUF partitions
MM_N = 512  # PE moving-operand max free dim

_cache = {}


def _tile_schedule(rows, f_tile, ramp, ramp_end=()):
    """Tile-size schedule: optional small prologue/epilogue tiles so the
    pipeline fills/drains quickly, f_tile-sized tiles in the middle."""
    head, tail = [], []
    left = rows
    for s in ramp:
        if left <= 0:
            break
        s = min(s, left)
        head.append(s)
        left -= s
    for s in ramp_end:
        if left <= 0:
            break
        s = min(s, left)
        tail.append(s)
        left -= s
    mid = []
    while left > 0:
        s = min(f_tile, left)
        mid.append(s)
        left -= s
    return head + mid + tail[::-1]


def _build(rows, sizes, in_bufs=5, out_bufs=4, tmp_bufs=3, psum_bufs=2,
           l2_engine="scalar", e1_engine="vector",
           store_engine="gpsimd", s_on_pe=True, c_dist=1,
           l2_dve_tiles=(), e1_gps_tiles=(), fuse_sq=False):
    """Per-core Bass module: input din [P, 3*rows] fp16 (tile-interleaved
    [d0h|d1h|lam] blocks), output dout [P, 2*rows] fp16 ([e0|e1] blocks)."""
    f16 = mybir.dt.float16
    f32 = mybir.dt.float32
    Act = mybir.ActivationFunctionType

    nc = bacc.Bacc(
        "TRN2",
        target_bir_lowering=False,
        debug=False,
        num_devices=N_CORES,
    )
    din = nc.dram_tensor("din", [P, 3 * rows], f16, kind="ExternalInput").ap()
    eye = nc.dram_tensor("eye", [P, P], f16, kind="ExternalInput").ap()
    dout = nc.dram_tensor("dout", [P, 2 * rows], f16, kind="ExternalOutput").ap()

    l2_eng = getattr(nc, l2_engine)
    e1_eng = getattr(nc, e1_engine)
    store_eng = getattr(nc, store_engine)

    with tile.TileContext(nc) as tc:
        with (
            tc.tile_pool(name="w", bufs=1) as wpool,
            tc.tile_pool(name="ins", bufs=in_bufs) as ins,
            tc.tile_pool(name="outs", bufs=out_bufs) as outs,
            tc.tile_pool(name="tmp", bufs=tmp_bufs) as tmp,
            tc.tile_pool(name="hpool", bufs=tmp_bufs + 2) as hpool,
            tc.tile_pool(name="ps", bufs=psum_bufs, space="PSUM") as ps,
        ):
            t_eye = wpool.tile([P, P], f16, tag="eye")
            if s_on_pe:
                nc.sync.dma_start(t_eye[:], eye)

            def stage_a(idx, f0, F):
                t_in = ins.tile([P, 3 * F], f16, tag="in")
                nc.sync.dma_start(t_in[:], din[:, 3 * f0 : 3 * f0 + 3 * F])
                t_d0 = t_in[:, 0:F]
                t_d1 = t_in[:, F : 2 * F]
                t_lam = t_in[:, 2 * F : 3 * F]

                if fuse_sq:
                    # h first (reads d1), then dif in place over d1's slot:
                    # the tile becomes [d0 | dif | lam], so ONE Square over
                    # the contiguous [dif | lam] 2F span yields [d2 | l2] --
                    # two squares for one op's fixed cost and semaphore set.
                    t_h = hpool.tile([P, F], f16, tag="h")
                    nc.vector.tensor_add(t_h[:], t_d0, t_d1)
                    nc.vector.tensor_sub(t_d1, t_d0, t_d1)
                    t_sq = tmp.tile([P, 2 * F], f16, tag="d2")
                    nc.scalar.activation(t_sq[:], t_in[:, F : 3 * F], Act.Square)
                    return {"idx": idx, "f0": f0, "F": F, "h": t_h,
                            "d2": t_sq[:, 0:F], "l2": t_sq[:, F : 2 * F]}

                # dif feeds the critical path (dif -> d2 -> s -> sqrt); the
                # l2 square reads lam straight from the packed input so it
                # can run as soon as the tile lands.
                t_dif = tmp.tile([P, F], f16, tag="dif")
                nc.vector.tensor_sub(t_dif[:], t_d0, t_d1)
                t_h = hpool.tile([P, F], f16, tag="h")
                nc.vector.tensor_add(t_h[:], t_d0, t_d1)

                t_d2 = tmp.tile([P, F], f16, tag="d2")
                nc.scalar.activation(t_d2[:], t_dif[:], Act.Square)
                t_l2 = tmp.tile([P, F], f16, tag="l2")
                # per-tile engine balancing: ACT is the pacing engine overall,
                # so a few tiles' lam^2 runs on DVE instead
                if idx in l2_dve_tiles:
                    nc.vector.tensor_mul(t_l2[:], t_lam, t_lam)
                elif l2_engine == "scalar":
                    nc.scalar.activation(t_l2[:], t_lam, Act.Square)
                else:
                    l2_eng.tensor_mul(t_l2[:], t_lam, t_lam)
                return {"idx": idx, "f0": f0, "F": F, "h": t_h,
                        "d2": t_d2, "l2": t_l2}

            def stage_b(st):
                F = st["F"]
                if s_on_pe:
                    p_s = ps.tile([P, F], f32, tag="s")
                    for c0 in range(0, F, MM_N):
                        w = min(MM_N, F - c0)
                        nc.tensor.matmul(
                            out=p_s[:, c0 : c0 + w], lhsT=t_eye[:],
                            rhs=st["d2"][:, c0 : c0 + w],
                            start=True, stop=False,
                        )
                        nc.tensor.matmul(
                            out=p_s[:, c0 : c0 + w], lhsT=t_eye[:],
                            rhs=st["l2"][:, c0 : c0 + w],
                            start=False, stop=True,
                        )
                    st["s"] = p_s
                else:
                    # accumulate in place over l2
                    nc.vector.tensor_add(st["l2"][:], st["d2"][:], st["l2"][:])
                    st["s"] = st["l2"]

            def stage_c(st):
                f0, F = st["f0"], st["F"]
                t_r = tmp.tile([P, F], f16, tag="r")
                nc.scalar.activation(t_r[:], st["s"][:], Act.Sqrt)
                t_out = outs.tile([P, 2 * F], f16, tag="out")
                nc.vector.tensor_sub(t_out[:, 0:F], st["h"][:], t_r[:])
                # off-load a few early tiles' e1 to the idle GpSimd to keep
                # DVE's total under the DMA pace
                e1e = nc.gpsimd if st["idx"] in e1_gps_tiles else e1_eng
                e1e.tensor_add(t_out[:, F : 2 * F], st["h"][:], t_r[:])
                store_eng.dma_start(dout[:, 2 * f0 : 2 * f0 + 2 * F], t_out[:])

            pend = []
            f0 = 0
            for idx, F in enumerate(sizes):
                pend.append(stage_a(idx, f0, F))
                if len(pend) >= 2:
                    stage_b(pend[-2])
                if len(pend) >= c_dist + 1:
                    stage_c(pend.pop(0))
                f0 += F
            for st in pend:
                if "s" not in st:
                    stage_b(st)
            for st in pend:
                stage_c(st)
    with _pin_act_table():
        nc.compile()
    return nc


def _get_nc(rows, sizes, **cfg):
    cfg = {k: (tuple(v) if isinstance(v, list) else v) for k, v in cfg.items()}
    key = (rows, tuple(sizes), tuple(sorted(cfg.items())))
    if key not in _cache:
        _cache[key] = _build(rows, sizes, **cfg)
    return _cache[key]


def kernel(d0, d1, lam, _trace=False, f_tile=2048, ramp=(512, 1024),
           ramp_end=(512,), **cfg):
    # 0.5*d0 and 0.5*d1 as the fp16 quantization scale: the device then
    # computes h/dif as plain adds with no halving op.
    d0 = (np.asarray(d0, dtype=np.float32) * 0.5).astype(np.float16).ravel()
    d1 = (np.asarray(d1, dtype=np.float32) * 0.5).astype(np.float16).ravel()
    lam = np.asarray(lam, dtype=np.float16).ravel()
    n = d0.shape[0]

    # Per-core sample count: multiple of 128, cores cover ceil(n / 8).
    rows = -(-n // (N_CORES * P))  # ceil
    C = P * rows
    total = N_CORES * C
    pad = total - n
    if pad:
        z = np.zeros(pad, np.float16)
        d0 = np.concatenate([d0, z])
        d1 = np.concatenate([d1, z])
        lam = np.concatenate([lam, z])

    sizes = _tile_schedule(rows, f_tile, tuple(ramp), tuple(ramp_end))
    bounds = np.cumsum([0] + sizes)

    eye = np.eye(P, dtype=np.float16)
    in_maps = []
    for c in range(N_CORES):
        sl = slice(c * C, (c + 1) * C)
        d0r = d0[sl].reshape(P, rows)
        d1r = d1[sl].reshape(P, rows)
        lamr = lam[sl].reshape(P, rows)
        din = np.empty((P, 3 * rows), np.float16)
        for F, f0 in zip(sizes, bounds):
            g = 3 * f0
            din[:, g : g + F] = d0r[:, f0 : f0 + F]
            din[:, g + F : g + 2 * F] = d1r[:, f0 : f0 + F]
            din[:, g + 2 * F : g + 3 * F] = lamr[:, f0 : f0 + F]
        in_maps.append({"din": din, "eye": eye})

    nc = _get_nc(rows, sizes, **cfg)
    res = run_bass_kernel_spmd(
        nc, in_maps, core_ids=list(range(N_CORES)), trace=_trace
    )
    global last_results
    last_results = res

    e0 = np.empty((N_CORES, P, rows), np.float16)
    e1 = np.empty((N_CORES, P, rows), np.float16)
    for c in range(N_CORES):
        outr = res.results[c]["dout"].reshape(P, 2 * rows)
        for F, f0 in zip(sizes, bounds):
            g = 2 * f0
            e0[c, :, f0 : f0 + F] = outr[:, g : g + F]
            e1[c, :, f0 : f0 + F] = outr[:, g + F : g + 2 * F]

    full = np.empty((n, 2), np.float32)
    full[:, 0] = e0.reshape(-1)[:n]
    full[:, 1] = e1.reshape(-1)[:n]
    return full


last_results = None


# revision 35
# speedup vs baseline: 1.0151x; 1.0151x over previous
# DiabaticReadout forward on Trainium2 (Bass/Tile), 8-core data-parallel.
#
# Per sample i: H = [[d0, lam], [lam, d1]] (2x2 symmetric).  Eigenvalues in
# closed form:
#   h = 0.5*(d0+d1);  r = sqrt(0.25*(d0-d1)^2 + lam^2);  e0, e1 = h -/+ r
# (ascending, matches eigh).
#
# Purely elementwise and HBM-bound, so the whole game is bytes: the harness
# gate is rel-err < 2e-2 against a ~7 output scale, while fp16 rounding of
# the streams costs ~1e-3 worst case.  Stream everything as fp16: 6 B/sample
# in + 4 B/sample out = 12.5 MB per core instead of 25 MB, a 2x cut in HBM
# traffic against the ~358 GB/s per-core HBM limit (~35 us floor).
#
# Layout: the host packs the three inputs tile-interleaved into ONE tensor
# ([d0-block | d1-block | lam-block] per [128, F] tile) and both outputs
# come back in one ([e0-block | e1-block]).  One dma_start per tile per
# direction with 12KB/8KB per-partition lines keeps the SDMA engines in
# their high-efficiency regime (separate fp16 tensors gave 4KB lines and
# 5 issues/tile) and the host does pure reshuffling.  d0,d1 are pre-scaled
# by 0.5 during the fp16 cast (a free quantization scale) so no on-device
# halving op is needed.
#
# Engine budget per [128, 2048] tile (~7.3 us of DMA, the pacer; ops run
# ~15% over their cost-model time when the SDMA engines are at full rate):
#   DVE    dif=d0h-d1h, h=d0h+d1h, e0=h-r, e1=h+r  (4 TT @ 2x fp16 ~5.4us)
#   ACT    d2=Square(dif), r=Sqrt(s_psum)          (2 passes ~4.5us)
#   GPSIMD l2=lam*lam (TT ~4us) + the output-store issue (SWDGE ring)
#   PE     s = I.d2 + I.l2 accumulated into PSUM f32 (8 id-matmuls ~4us);
#          the identity weights ship as a tiny extra input
#   Sync   the input-load issue (SP HWDGE ring)
# scalar_tensor_tensor is avoided (only a 1x DVE uop; plain tensor_tensor
# runs 2x on fp16) and both ACT functions live in the single
# sqrt_and_others table so there is exactly one ACT_TABLE_LOAD.
#
# The Tile scheduler keeps per-engine program order, so emitting a tile's
# whole chain at once would make the in-order ACT/DVE streams block
# mid-chain on the PE adder / GPSIMD's lam^2 every tile.  Each Python
# iteration instead emits a 3-stage software pipeline
#   A(i): load, dif/h, d2, l2    B(i-1): PE matmul-add    C(i-2): sqrt,
#   e0/e1, store
# so every op's inputs finished a full tile-period earlier and no engine
# ever blocks inside its stream.

import numpy as np

import concourse.bacc as bacc_mod
import concourse.tile as tile
from concourse import bacc, mybir
from concourse.bass_utils import run_bass_kernel_spmd

import contextlib


@contextlib.contextmanager
def _pin_act_table(keep="sqrt_and_others"):
    """Square and Sqrt both live in the `sqrt_and_others` set, but the
    table-load pass greedily picks the first set containing each function,
    which alternates tables per tile (~2.5us/tile of ACT_TABLE_LOAD
    thrash).  Present every other set as empty during compile so the pass
    pins everything to one table; indices stay aligned with act_info.json."""
    orig = bacc_mod.get_activation_tables

    def patched(arch):
        t = orig(arch)
        assert keep in t, sorted(t)
        return {name: (funcs if name == keep else set()) for name, funcs in t.items()}

    bacc_mod.get_activation_tables = patched
    try:
        yield
    finally:
        bacc_mod.get_activation_tables = orig

N_CORES = 8
P = 128  # SBUF partitions
MM_N = 512  # PE moving-operand max free dim

_cache = {}


def _tile_schedule(rows, f_tile, ramp, ramp_end=()):
    """Tile-size schedule: optional small prologue/epilogue tiles so the
    pipeline fills/drains quickly, f_tile-sized tiles in the middle."""
    head, tail = [], []
    left = rows
    for s in ramp:
        if left <= 0:
            break
        s = min(s, left)
        head.append(s)
        left -= s
    for s in ramp_end:
        if left <= 0:
            break
        s = min(s, left)
        tail.append(s)
        left -= s
    mid = []
    while left > 0:
        s = min(f_tile, left)
        mid.append(s)
        left -= s
    return head + mid + tail[::-1]


def _build(rows, sizes, in_bufs=5, out_bufs=4, tmp_bufs=3, psum_bufs=2,
           l2_engine="scalar", e1_engine="vector",
           store_engine="gpsimd", s_on_pe=True, c_dist=1,
           l2_dve_tiles=(), e1_gps_tiles=(), fuse_sq=False, split_store=False):
    """Per-core Bass module: input din [P, 3*rows] fp16 (tile-interleaved
    [d0h|d1h|lam] blocks), output dout [P, 2*rows] fp16 ([e0|e1] blocks)."""
    f16 = mybir.dt.float16
    f32 = mybir.dt.float32
    Act = mybir.ActivationFunctionType

    nc = bacc.Bacc(
        "TRN2",
        target_bir_lowering=False,
        debug=False,
        num_devices=N_CORES,
    )
    din = nc.dram_tensor("din", [P, 3 * rows], f16, kind="ExternalInput").ap()
    eye = nc.dram_tensor("eye", [P, P], f16, kind="ExternalInput").ap()
    dout = nc.dram_tensor("dout", [P, 2 * rows], f16, kind="ExternalOutput").ap()

    l2_eng = getattr(nc, l2_engine)
    e1_eng = getattr(nc, e1_engine)
    store_eng = getattr(nc, store_engine)

    with tile.TileContext(nc) as tc:
        with (
            tc.tile_pool(name="w", bufs=1) as wpool,
            tc.tile_pool(name="ins", bufs=in_bufs) as ins,
            tc.tile_pool(name="outs", bufs=out_bufs) as outs,
            tc.tile_pool(name="tmp", bufs=tmp_bufs) as tmp,
            tc.tile_pool(name="hpool", bufs=tmp_bufs + 2) as hpool,
            tc.tile_pool(name="ps", bufs=psum_bufs, space="PSUM") as ps,
        ):
            t_eye = wpool.tile([P, P], f16, tag="eye")
            if s_on_pe:
                nc.sync.dma_start(t_eye[:], eye)

            def stage_a(idx, f0, F):
                t_in = ins.tile([P, 3 * F], f16, tag="in")
                nc.sync.dma_start(t_in[:], din[:, 3 * f0 : 3 * f0 + 3 * F])
                t_d0 = t_in[:, 0:F]
                t_d1 = t_in[:, F : 2 * F]
                t_lam = t_in[:, 2 * F : 3 * F]

                if fuse_sq:
                    # h first (reads d1), then dif in place over d1's slot:
                    # the tile becomes [d0 | dif | lam], so ONE Square over
                    # the contiguous [dif | lam] 2F span yields [d2 | l2] --
                    # two squares for one op's fixed cost and semaphore set.
                    t_h = hpool.tile([P, F], f16, tag="h")
                    nc.vector.tensor_add(t_h[:], t_d0, t_d1)
                    nc.vector.tensor_sub(t_d1, t_d0, t_d1)
                    t_sq = tmp.tile([P, 2 * F], f16, tag="d2")
                    nc.scalar.activation(t_sq[:], t_in[:, F : 3 * F], Act.Square)
                    return {"idx": idx, "f0": f0, "F": F, "h": t_h,
                            "d2": t_sq[:, 0:F], "l2": t_sq[:, F : 2 * F]}

                # dif feeds the critical path (dif -> d2 -> s -> sqrt); the
                # l2 square reads lam straight from the packed input so it
                # can run as soon as the tile lands.
                t_dif = tmp.tile([P, F], f16, tag="dif")
                nc.vector.tensor_sub(t_dif[:], t_d0, t_d1)
                t_h = hpool.tile([P, F], f16, tag="h")
                nc.vector.tensor_add(t_h[:], t_d0, t_d1)

                t_d2 = tmp.tile([P, F], f16, tag="d2")
                nc.scalar.activation(t_d2[:], t_dif[:], Act.Square)
                t_l2 = tmp.tile([P, F], f16, tag="l2")
                # per-tile engine balancing: ACT is the pacing engine overall,
                # so a few tiles' lam^2 runs on DVE instead
                if idx in l2_dve_tiles:
                    nc.vector.tensor_mul(t_l2[:], t_lam, t_lam)
                elif l2_engine == "scalar":
                    nc.scalar.activation(t_l2[:], t_lam, Act.Square)
                else:
                    l2_eng.tensor_mul(t_l2[:], t_lam, t_lam)
                return {"idx": idx, "f0": f0, "F": F, "h": t_h,
                        "d2": t_d2, "l2": t_l2}

            def stage_b(st):
                F = st["F"]
                if s_on_pe:
                    p_s = ps.tile([P, F], f32, tag="s")
                    for c0 in range(0, F, MM_N):
                        w = min(MM_N, F - c0)
                        nc.tensor.matmul(
                            out=p_s[:, c0 : c0 + w], lhsT=t_eye[:],
                            rhs=st["d2"][:, c0 : c0 + w],
                            start=True, stop=False,
                        )
                        nc.tensor.matmul(
                            out=p_s[:, c0 : c0 + w], lhsT=t_eye[:],
                            rhs=st["l2"][:, c0 : c0 + w],
                            start=False, stop=True,
                        )
                    st["s"] = p_s
                else:
                    # accumulate in place over l2
                    nc.vector.tensor_add(st["l2"][:], st["d2"][:], st["l2"][:])
                    st["s"] = st["l2"]

            def stage_c(st):
                f0, F = st["f0"], st["F"]
                t_r = tmp.tile([P, F], f16, tag="r")
                nc.scalar.activation(t_r[:], st["s"][:], Act.Sqrt)
                t_out = outs.tile([P, 2 * F], f16, tag="out")
                nc.vector.tensor_sub(t_out[:, 0:F], st["h"][:], t_r[:])
                if split_store:
                    # ship the e0 half as soon as it exists: smaller lines,
                    # but the store stream starts ~1.2us earlier per tile
                    store_eng.dma_start(
                        dout[:, 2 * f0 : 2 * f0 + F], t_out[:, 0:F]
                    )
                # off-load a few early tiles' e1 to the idle GpSimd to keep
                # DVE's total under the DMA pace
                e1e = nc.gpsimd if st["idx"] in e1_gps_tiles else e1_eng
                e1e.tensor_add(t_out[:, F : 2 * F], st["h"][:], t_r[:])
                if split_store:
                    store_eng.dma_start(
                        dout[:, 2 * f0 + F : 2 * f0 + 2 * F], t_out[:, F : 2 * F]
                    )
                else:
                    store_eng.dma_start(dout[:, 2 * f0 : 2 * f0 + 2 * F], t_out[:])

            pend = []
            f0 = 0
            for idx, F in enumerate(sizes):
                pend.append(stage_a(idx, f0, F))
                if len(pend) >= 2:
                    stage_b(pend[-2])
                if len(pend) >= c_dist + 1:
                    stage_c(pend.pop(0))
                f0 += F
            for st in pend:
                if "s" not in st:
                    stage_b(st)
            for st in pend:
                stage_c(st)
    with _pin_act_table():
        nc.compile()
    return nc


def _get_nc(rows, sizes, **cfg):
    cfg = {k: (tuple(v) if isinstance(v, list) else v) for k, v in cfg.items()}
    key = (rows, tuple(sizes), tuple(sorted(cfg.items())))
    if key not in _cache:
        _cache[key] = _build(rows, sizes, **cfg)
    return _cache[key]


def kernel(d0, d1, lam, _trace=False, f_tile=2048, ramp=(512, 1024),
           ramp_end=(512,), **cfg):
    # 0.5*d0 and 0.5*d1 as the fp16 quantization scale: the device then
    # computes h/dif as plain adds with no halving op.
    d0 = (np.asarray(d0, dtype=np.float32) * 0.5).astype(np.float16).ravel()
    d1 = (np.asarray(d1, dtype=np.float32) * 0.5).astype(np.float16).ravel()
    lam = np.asarray(lam, dtype=np.float16).ravel()
    n = d0.shape[0]

    # Per-core sample count: multiple of 128, cores cover ceil(n / 8).
    rows = -(-n // (N_CORES * P))  # ceil
    C = P * rows
    total = N_CORES * C
    pad = total - n
    if pad:
        z = np.zeros(pad, np.float16)
        d0 = np.concatenate([d0, z])
        d1 = np.concatenate([d1, z])
        lam = np.concatenate([lam, z])

    sizes = _tile_schedule(rows, f_tile, tuple(ramp), tuple(ramp_end))
    bounds = np.cumsum([0] + sizes)

    eye = np.eye(P, dtype=np.float16)
    in_maps = []
    for c in range(N_CORES):
        sl = slice(c * C, (c + 1) * C)
        d0r = d0[sl].reshape(P, rows)
        d1r = d1[sl].reshape(P, rows)
        lamr = lam[sl].reshape(P, rows)
        din = np.empty((P, 3 * rows), np.float16)
        for F, f0 in zip(sizes, bounds):
            g = 3 * f0
            din[:, g : g + F] = d0r[:, f0 : f0 + F]
            din[:, g + F : g + 2 * F] = d1r[:, f0 : f0 + F]
            din[:, g + 2 * F : g + 3 * F] = lamr[:, f0 : f0 + F]
        in_maps.append({"din": din, "eye": eye})

    nc = _get_nc(rows, sizes, **cfg)
    res = run_bass_kernel_spmd(
        nc, in_maps, core_ids=list(range(N_CORES)), trace=_trace
    )
    global last_results
    last_results = res

    e0 = np.empty((N_CORES, P, rows), np.float16)
    e1 = np.empty((N_CORES, P, rows), np.float16)
    for c in range(N_CORES):
        outr = res.results[c]["dout"].reshape(P, 2 * rows)
        for F, f0 in zip(sizes, bounds):
            g = 2 * f0
            e0[c, :, f0 : f0 + F] = outr[:, g : g + F]
            e1[c, :, f0 : f0 + F] = outr[:, g + F : g + 2 * F]

    full = np.empty((n, 2), np.float32)
    full[:, 0] = e0.reshape(-1)[:n]
    full[:, 1] = e1.reshape(-1)[:n]
    return full


last_results = None


# revision 37
# speedup vs baseline: 1.0157x; 1.0005x over previous
# DiabaticReadout forward on Trainium2 (Bass/Tile), 8-core data-parallel.
#
# Per sample i: H = [[d0, lam], [lam, d1]] (2x2 symmetric).  Eigenvalues in
# closed form:
#   h = 0.5*(d0+d1);  r = sqrt(0.25*(d0-d1)^2 + lam^2);  e0, e1 = h -/+ r
# (ascending, matches eigh).
#
# Purely elementwise and HBM-bound, so the whole game is bytes: the harness
# gate is rel-err < 2e-2 against a ~7 output scale, while fp16 rounding of
# the streams costs ~1e-3 worst case.  Stream everything as fp16: 6 B/sample
# in + 4 B/sample out = 12.5 MB per core instead of 25 MB, a 2x cut in HBM
# traffic against the ~358 GB/s per-core HBM limit (~35 us floor).
#
# Layout: the host packs the three inputs tile-interleaved into ONE tensor
# ([d0-block | d1-block | lam-block] per [128, F] tile) and both outputs
# come back in one ([e0-block | e1-block]).  One dma_start per tile per
# direction with 12KB/8KB per-partition lines keeps the SDMA engines in
# their high-efficiency regime (separate fp16 tensors gave 4KB lines and
# 5 issues/tile) and the host does pure reshuffling.  d0,d1 are pre-scaled
# by 0.5 during the fp16 cast (a free quantization scale) so no on-device
# halving op is needed.
#
# Engine budget per [128, 2048] tile (~7.3 us of DMA, the pacer; ops run
# ~15% over their cost-model time when the SDMA engines are at full rate):
#   DVE    dif=d0h-d1h, h=d0h+d1h, e0=h-r, e1=h+r  (4 TT @ 2x fp16 ~5.4us)
#   ACT    d2=Square(dif), r=Sqrt(s_psum)          (2 passes ~4.5us)
#   GPSIMD l2=lam*lam (TT ~4us) + the output-store issue (SWDGE ring)
#   PE     s = I.d2 + I.l2 accumulated into PSUM f32 (8 id-matmuls ~4us);
#          the identity weights ship as a tiny extra input
#   Sync   the input-load issue (SP HWDGE ring)
# scalar_tensor_tensor is avoided (only a 1x DVE uop; plain tensor_tensor
# runs 2x on fp16) and both ACT functions live in the single
# sqrt_and_others table so there is exactly one ACT_TABLE_LOAD.
#
# The Tile scheduler keeps per-engine program order, so emitting a tile's
# whole chain at once would make the in-order ACT/DVE streams block
# mid-chain on the PE adder / GPSIMD's lam^2 every tile.  Each Python
# iteration instead emits a 3-stage software pipeline
#   A(i): load, dif/h, d2, l2    B(i-1): PE matmul-add    C(i-2): sqrt,
#   e0/e1, store
# so every op's inputs finished a full tile-period earlier and no engine
# ever blocks inside its stream.

import numpy as np

import concourse.bacc as bacc_mod
import concourse.tile as tile
from concourse import bacc, mybir
from concourse.bass_utils import run_bass_kernel_spmd

import contextlib


@contextlib.contextmanager
def _pin_act_table(keep="sqrt_and_others"):
    """Square and Sqrt both live in the `sqrt_and_others` set, but the
    table-load pass greedily picks the first set containing each function,
    which alternates tables per tile (~2.5us/tile of ACT_TABLE_LOAD
    thrash).  Present every other set as empty during compile so the pass
    pins everything to one table; indices stay aligned with act_info.json."""
    orig = bacc_mod.get_activation_tables

    def patched(arch):
        t = orig(arch)
        assert keep in t, sorted(t)
        return {name: (funcs if name == keep else set()) for name, funcs in t.items()}

    bacc_mod.get_activation_tables = patched
    try:
        yield
    finally:
        bacc_mod.get_activation_tables = orig

N_CORES = 8
P = 128  # SBUF partitions
MM_N = 512  # PE moving-operand max free dim

_cache = {}


def _tile_schedule(rows, f_tile, ramp, ramp_end=()):
    """Tile-size schedule: optional small prologue/epilogue tiles so the
    pipeline fills/drains quickly, f_tile-sized tiles in the middle."""
    head, tail = [], []
    left = rows
    for s in ramp:
        if left <= 0:
            break
        s = min(s, left)
        head.append(s)
        left -= s
    for s in ramp_end:
        if left <= 0:
            break
        s = min(s, left)
        tail.append(s)
        left -= s
    mid = []
    while left > 0:
        s = min(f_tile, left)
        mid.append(s)
        left -= s
    return head + mid + tail[::-1]


def _build(rows, sizes, in_bufs=5, out_bufs=4, tmp_bufs=3, psum_bufs=2,
           l2_engine="scalar", e1_engine="vector",
           store_engine="gpsimd", s_on_pe=True, c_dist=1,
           l2_dve_tiles=(), e1_gps_tiles=(), fuse_sq=False, split_store=False,
           r_in_out=False):
    """Per-core Bass module: input din [P, 3*rows] fp16 (tile-interleaved
    [d0h|d1h|lam] blocks), output dout [P, 2*rows] fp16 ([e0|e1] blocks)."""
    f16 = mybir.dt.float16
    f32 = mybir.dt.float32
    Act = mybir.ActivationFunctionType

    nc = bacc.Bacc(
        "TRN2",
        target_bir_lowering=False,
        debug=False,
        num_devices=N_CORES,
    )
    din = nc.dram_tensor("din", [P, 3 * rows], f16, kind="ExternalInput").ap()
    eye = nc.dram_tensor("eye", [P, P], f16, kind="ExternalInput").ap()
    dout = nc.dram_tensor("dout", [P, 2 * rows], f16, kind="ExternalOutput").ap()

    l2_eng = getattr(nc, l2_engine)
    e1_eng = getattr(nc, e1_engine)
    store_eng = getattr(nc, store_engine)

    with tile.TileContext(nc) as tc:
        with (
            tc.tile_pool(name="w", bufs=1) as wpool,
            tc.tile_pool(name="ins", bufs=in_bufs) as ins,
            tc.tile_pool(name="outs", bufs=out_bufs) as outs,
            tc.tile_pool(name="tmp", bufs=tmp_bufs) as tmp,
            tc.tile_pool(name="hpool", bufs=tmp_bufs + 2) as hpool,
            tc.tile_pool(name="ps", bufs=psum_bufs, space="PSUM") as ps,
        ):
            t_eye = wpool.tile([P, P], f16, tag="eye")
            if s_on_pe:
                nc.sync.dma_start(t_eye[:], eye)

            def stage_a(idx, f0, F):
                t_in = ins.tile([P, 3 * F], f16, tag="in")
                nc.sync.dma_start(t_in[:], din[:, 3 * f0 : 3 * f0 + 3 * F])
                t_d0 = t_in[:, 0:F]
                t_d1 = t_in[:, F : 2 * F]
                t_lam = t_in[:, 2 * F : 3 * F]

                if fuse_sq:
                    # h first (reads d1), then dif in place over d1's slot:
                    # the tile becomes [d0 | dif | lam], so ONE Square over
                    # the contiguous [dif | lam] 2F span yields [d2 | l2] --
                    # two squares for one op's fixed cost and semaphore set.
                    t_h = hpool.tile([P, F], f16, tag="h")
                    nc.vector.tensor_add(t_h[:], t_d0, t_d1)
                    nc.vector.tensor_sub(t_d1, t_d0, t_d1)
                    t_sq = tmp.tile([P, 2 * F], f16, tag="d2")
                    nc.scalar.activation(t_sq[:], t_in[:, F : 3 * F], Act.Square)
                    return {"idx": idx, "f0": f0, "F": F, "h": t_h,
                            "d2": t_sq[:, 0:F], "l2": t_sq[:, F : 2 * F]}

                # dif feeds the critical path (dif -> d2 -> s -> sqrt); the
                # l2 square reads lam straight from the packed input so it
                # can run as soon as the tile lands.
                t_dif = tmp.tile([P, F], f16, tag="dif")
                nc.vector.tensor_sub(t_dif[:], t_d0, t_d1)
                t_h = hpool.tile([P, F], f16, tag="h")
                nc.vector.tensor_add(t_h[:], t_d0, t_d1)

                t_d2 = tmp.tile([P, F], f16, tag="d2")
                nc.scalar.activation(t_d2[:], t_dif[:], Act.Square)
                t_l2 = tmp.tile([P, F], f16, tag="l2")
                # per-tile engine balancing: ACT is the pacing engine overall,
                # so a few tiles' lam^2 runs on DVE instead
                if idx in l2_dve_tiles:
                    nc.vector.tensor_mul(t_l2[:], t_lam, t_lam)
                elif l2_engine == "scalar":
                    nc.scalar.activation(t_l2[:], t_lam, Act.Square)
                else:
                    l2_eng.tensor_mul(t_l2[:], t_lam, t_lam)
                return {"idx": idx, "f0": f0, "F": F, "h": t_h,
                        "d2": t_d2, "l2": t_l2}

            def stage_b(st):
                F = st["F"]
                if s_on_pe:
                    p_s = ps.tile([P, F], f32, tag="s")
                    for c0 in range(0, F, MM_N):
                        w = min(MM_N, F - c0)
                        nc.tensor.matmul(
                            out=p_s[:, c0 : c0 + w], lhsT=t_eye[:],
                            rhs=st["d2"][:, c0 : c0 + w],
                            start=True, stop=False,
                        )
                        nc.tensor.matmul(
                            out=p_s[:, c0 : c0 + w], lhsT=t_eye[:],
                            rhs=st["l2"][:, c0 : c0 + w],
                            start=False, stop=True,
                        )
                    st["s"] = p_s
                else:
                    # accumulate in place over l2
                    nc.vector.tensor_add(st["l2"][:], st["d2"][:], st["l2"][:])
                    st["s"] = st["l2"]

            def stage_c(st):
                f0, F = st["f0"], st["F"]
                t_out = outs.tile([P, 2 * F], f16, tag="out")
                if r_in_out:
                    # sqrt lands straight in the e1 slot; e0 reads it, then
                    # e1 = h + r overwrites it in place -- one fewer tile
                    # (and its semaphore edges) per iteration
                    r_ap = t_out[:, F : 2 * F]
                    nc.scalar.activation(r_ap, st["s"][:], Act.Sqrt)
                    nc.vector.tensor_sub(t_out[:, 0:F], st["h"][:], r_ap)
                    nc.vector.tensor_add(r_ap, st["h"][:], r_ap)
                    store_eng.dma_start(
                        dout[:, 2 * f0 : 2 * f0 + 2 * F], t_out[:]
                    )
                    return
                t_r = tmp.tile([P, F], f16, tag="r")
                nc.scalar.activation(t_r[:], st["s"][:], Act.Sqrt)
                nc.vector.tensor_sub(t_out[:, 0:F], st["h"][:], t_r[:])
                if split_store:
                    # ship the e0 half as soon as it exists: smaller lines,
                    # but the store stream starts ~1.2us earlier per tile
                    store_eng.dma_start(
                        dout[:, 2 * f0 : 2 * f0 + F], t_out[:, 0:F]
                    )
                # off-load a few early tiles' e1 to the idle GpSimd to keep
                # DVE's total under the DMA pace
                e1e = nc.gpsimd if st["idx"] in e1_gps_tiles else e1_eng
                e1e.tensor_add(t_out[:, F : 2 * F], st["h"][:], t_r[:])
                if split_store:
                    store_eng.dma_start(
                        dout[:, 2 * f0 + F : 2 * f0 + 2 * F], t_out[:, F : 2 * F]
                    )
                else:
                    store_eng.dma_start(dout[:, 2 * f0 : 2 * f0 + 2 * F], t_out[:])

            pend = []
            f0 = 0
            for idx, F in enumerate(sizes):
                pend.append(stage_a(idx, f0, F))
                if len(pend) >= 2:
                    stage_b(pend[-2])
                if len(pend) >= c_dist + 1:
                    stage_c(pend.pop(0))
                f0 += F
            for st in pend:
                if "s" not in st:
                    stage_b(st)
            for st in pend:
                stage_c(st)
    with _pin_act_table():
        nc.compile()
    return nc


def _get_nc(rows, sizes, **cfg):
    cfg = {k: (tuple(v) if isinstance(v, list) else v) for k, v in cfg.items()}
    key = (rows, tuple(sizes), tuple(sorted(cfg.items())))
    if key not in _cache:
        _cache[key] = _build(rows, sizes, **cfg)
    return _cache[key]


def kernel(d0, d1, lam, _trace=False, f_tile=2048, ramp=(512, 1024),
           ramp_end=(512,), **cfg):
    # 0.5*d0 and 0.5*d1 as the fp16 quantization scale: the device then
    # computes h/dif as plain adds with no halving op.
    d0 = (np.asarray(d0, dtype=np.float32) * 0.5).astype(np.float16).ravel()
    d1 = (np.asarray(d1, dtype=np.float32) * 0.5).astype(np.float16).ravel()
    lam = np.asarray(lam, dtype=np.float16).ravel()
    n = d0.shape[0]

    # Per-core sample count: multiple of 128, cores cover ceil(n / 8).
    rows = -(-n // (N_CORES * P))  # ceil
    C = P * rows
    total = N_CORES * C
    pad = total - n
    if pad:
        z = np.zeros(pad, np.float16)
        d0 = np.concatenate([d0, z])
        d1 = np.concatenate([d1, z])
        lam = np.concatenate([lam, z])

    sizes = _tile_schedule(rows, f_tile, tuple(ramp), tuple(ramp_end))
    bounds = np.cumsum([0] + sizes)

    eye = np.eye(P, dtype=np.float16)
    in_maps = []
    for c in range(N_CORES):
        sl = slice(c * C, (c + 1) * C)
        d0r = d0[sl].reshape(P, rows)
        d1r = d1[sl].reshape(P, rows)
        lamr = lam[sl].reshape(P, rows)
        din = np.empty((P, 3 * rows), np.float16)
        for F, f0 in zip(sizes, bounds):
            g = 3 * f0
            din[:, g : g + F] = d0r[:, f0 : f0 + F]
            din[:, g + F : g + 2 * F] = d1r[:, f0 : f0 + F]
            din[:, g + 2 * F : g + 3 * F] = lamr[:, f0 : f0 + F]
        in_maps.append({"din": din, "eye": eye})

    nc = _get_nc(rows, sizes, **cfg)
    res = run_bass_kernel_spmd(
        nc, in_maps, core_ids=list(range(N_CORES)), trace=_trace
    )
    global last_results
    last_results = res

    e0 = np.empty((N_CORES, P, rows), np.float16)
    e1 = np.empty((N_CORES, P, rows), np.float16)
    for c in range(N_CORES):
        outr = res.results[c]["dout"].reshape(P, 2 * rows)
        for F, f0 in zip(sizes, bounds):
            g = 2 * f0
            e0[c, :, f0 : f0 + F] = outr[:, g : g + F]
            e1[c, :, f0 : f0 + F] = outr[:, g + F : g + 2 * F]

    full = np.empty((n, 2), np.float32)
    full[:, 0] = e0.reshape(-1)[:n]
    full[:, 1] = e1.reshape(-1)[:n]
    return full


last_results = None
